# revision 15
# baseline (speedup 1.0000x reference)
# CondConv2d Trainium2 kernel (Bass/Tile), data-parallel over batch on 8 cores.
#
# Problem (hardcoded):
#   x:               [32, 256, 56, 56] f32
#   routing_weights: [32, 8] f32
#   weight_experts:  [8, 589824] f32      (589824 = 256*256*3*3, co-major)
#   out:             [32, 256, 56, 56] f32
#   out[b] = conv2d(x[b], (routing[b] @ experts).reshape(256,256,3,3), pad=1)
#
# Sharding: 4 samples per core; expert bank replicated.
#
# Per-core kernel (pipelined):
#   M1 (interleaved mix): natural-layout mixed weights for all 4 samples via
#     matmuls with a block-diagonal routing lhsT  L[(e,j'),(s,j)] = r[s,e]*I32,
#     contraction over (4 experts x 32 co-rows); two accumulating matmuls
#     (expert groups 0-3 / 4-7) per psum tile.  8 co-groups (cg) of 32 rows.
#   M2 (transpose): per (cg, ci-chunk, tap): one identity-rhs matmul
#     transposes the (s,j)-packed natural rows into [ci, (s,j)] columns;
#     strided DVE copies scatter them into per-sample conv lhsT layout.
#   Conv: 9 taps x 2 ci-chunks bf16 matmuls (N=448, 8-row bands) into PSUM
#     over zero-padded [128, 58*58] bf16 input tiles; (tap, ci-chunk) is the
#     outer loop over 4 concurrent band psums so consecutive matmuls share
#     the stationary operand (fewer LDWEIGHTS reloads).
#   Data movement (measured per-queue rates differ ~2x): the expert bank
#     streams as f32->bf16 cast DMAs on the gpsimd/SWDGE queue ordered
#     cg0..3 (conv(0,0)'s weights) then x0 then cg4..7; x rides f32 staging
#     tiles + deferred DVE cast/pad copies (x1..x3 on the HWDGE queue);
#     conv jobs run b-major (all samples' co-half 0 first) so the second
#     half of the expert bank has maximal slack; outputs drain on HWDGE.
#   A consumer of DMA X transitively waits for all earlier DMAs on X's
#   queue, so per-queue emission order is need-order.

import numpy as np

import concourse.bass as bass
import concourse.mybir as mybir
import concourse.tile as tile
from concourse import bacc, bass_utils

# ---- problem constants ----
B = 32
E = 8
C = 256  # CIN = COUT
H = W = 56
KH = KW = 3
NPARAM = C * C * KH * KW  # 589824
N_CORES = 8
B_LOC = B // N_CORES  # 4 samples per core

PW = W + 2  # 58
PHW = PW * PW  # 3364
CI_T = C * KH * KW  # 2304  (ci*9 + t) extent per co row
NCHUNK = C // 128  # 2 chunks of 128 for both ci and co
ROWS = 8  # output rows per conv matmul tile
NSP = H // ROWS  # 7 spatial tiles
NTAP = KH * KW  # 9
NCG = 8  # co-groups of 32 rows
CGR = 32  # rows per co-group

F32 = mybir.dt.float32
BF16 = mybir.dt.bfloat16

X0_VIA = "poolstage"  # how x[0] is loaded: pool | sync | poolstage
PROBE = None  # None | "conv" | "mix"
M1_BUFS = 2
M2_BUFS = 2
BAND_MAX = 8
BANDS = [(i * 8, 8) for i in range(7)]
BAND_GRP = 4  # bands accumulated concurrently (shared stationary weights)


def build_program(reps=1, hw_loop=False):
    nc = bacc.Bacc(
        "TRN2",
        target_bir_lowering=False,
        debug=False,
        enable_asserts=False,
    )

    x_d = nc.dram_tensor("x_loc", [B_LOC, C, H, W], F32, kind="ExternalInput").ap()
    r_d = nc.dram_tensor("r_loc", [B_LOC, E], F32, kind="ExternalInput").ap()
    w_d = nc.dram_tensor("experts", [E, NPARAM], F32, kind="ExternalInput").ap()
    o_d = nc.dram_tensor("out_loc", [B_LOC, C, H, W], F32, kind="ExternalOutput").ap()

    with tile.TileContext(nc) as tc:
        if hw_loop and reps > 1:
            with tc.For_i(0, reps):
                _emit_body(nc, tc, x_d, r_d, w_d, o_d)
        else:
            for _ in range(reps):
                _emit_body(nc, tc, x_d, r_d, w_d, o_d)

    nc.compile()
    return nc


def _emit_body(nc, tc, x_d, r_d, w_d, o_d):
    w_v = w_d.rearrange("e (co r) -> e co r", r=CI_T)  # [8, 256, 2304]

    with (
        tc.tile_pool(name="const", bufs=1) as cpool,
        tc.tile_pool(name="wexp", bufs=8) as wepool,
        tc.tile_pool(name="wef32", bufs=4) as wef32pool,
        tc.tile_pool(name="stg", bufs=2) as stgpool,
        tc.tile_pool(name="xpad", bufs=8) as xppool,
        tc.tile_pool(name="osb", bufs=7) as opool,
        tc.tile_pool(name="ps", bufs=3, space="PSUM") as ps,
        tc.tile_pool(name="cv", bufs=BAND_GRP, space="PSUM") as pcv,
    ):
        # ---- prep: broadcast routing weights to all partitions ----
        r_sb = cpool.tile([1, B_LOC * E], F32, tag="r_sb")
        nc.sync.dma_start(out=r_sb, in_=r_d.rearrange("s e -> (s e)")[None, :])
        ones = cpool.tile([1, 128], F32, tag="ones")
        nc.vector.memset(ones, 1.0)
        r_ps = ps.tile([128, B_LOC * E], F32, tag="m2", bufs=M2_BUFS)
        nc.tensor.matmul(r_ps, lhsT=ones, rhs=r_sb, start=True, stop=True)
        r_bc = cpool.tile([128, B_LOC * E], F32, tag="r_bc")
        nc.scalar.copy(r_bc, r_ps)

        # ---- identity (bf16) ----
        ident = cpool.tile([128, 128], BF16, tag="ident")
        nc.gpsimd.memset(ident, 0.0)
        nc.gpsimd.affine_select(
            out=ident,
            in_=ident,
            compare_op=mybir.AluOpType.not_equal,
            fill=1.0,
            base=0,
            pattern=[[-1, 128]],
            channel_multiplier=1,
        )

        # ---- block-diagonal routing lhsT for the interleaved mix ----
        # L[g][e_rel*32+j', s*32+j] = r[s, g*4+e_rel] * (j==j')
        Ls, Ls32 = [], []
        for g in range(2):
            for dt, lst in ((BF16, Ls), (F32, Ls32)):
                sfx = "" if dt is BF16 else "f"
                L_g = cpool.tile([128, 128], dt, tag=f"L{g}{sfx}", name=f"L_g{g}{sfx}")
                nc.vector.memset(L_g, 0.0)
                for er in range(4):
                    pr = slice(er * CGR, (er + 1) * CGR)
                    for s in range(B_LOC):
                        c = s * E + g * 4 + er
                        nc.vector.tensor_scalar_mul(
                            L_g[pr, s * CGR : (s + 1) * CGR],
                            ident[pr, pr.start : pr.stop],
                            r_bc[pr, c : c + 1],
                        )
                lst.append(L_g)

        # per-sample transposed mixed weights, conv lhsT layout:
        # wmT[b][:, s*2304 + (a*9+t)*128 + co'] = W_s[b*128+co', a*128+ci, t]
        wmT = []
        for b in range(NCHUNK):
            wt = cpool.tile([128, B_LOC * CI_T], BF16, tag=f"wmT{b}")
            wmT.append(wt)
        # [128, 18, 4, 128] views: (a*9+t, s, co')
        wmT_v = [
            wt.rearrange("p (s at c) -> p at s c", s=B_LOC, at=NCHUNK * NTAP, c=128)
            for wt in wmT
        ]

        stg = [None] * NCG
        xpads = [None] * B_LOC

        def emit_expert_dma(cg, via="pool"):
            """DMA expert rows cg*32..cg*32+32 for both expert groups, bf16.

            via="pool": gpsimd cast DMA (SWDGE queue).
            via="sync": f32 DMA on the HWDGE queue + DVE cast copy, so the
            expert stream is split across two DMA queues.
            """
            tiles = []
            for g in range(2):
                we = wepool.tile([128, CI_T], BF16, tag="wexp", name=f"we{cg}_{g}")
                src = w_v[g * 4 : (g + 1) * 4, cg * CGR : (cg + 1) * CGR, :]
                if via == "pool":
                    nc.gpsimd.dma_start(out=we, in_=src)  # f32 -> bf16 cast DMA
                elif via == "poolf32":
                    # plain f32 load (full DMA rate); M1 consumes f32 directly
                    wf = wef32pool.tile(
                        [128, CI_T], F32, tag="wef32", name=f"wef{cg}_{g}"
                    )
                    nc.gpsimd.dma_start(out=wf, in_=src)
                    tiles.append(wf)
                    continue
                else:
                    wf = wef32pool.tile(
                        [128, CI_T], F32, tag="wef32", name=f"wef{cg}_{g}"
                    )
                    eng = nc.scalar if via == "scalar" else nc.sync
                    eng.dma_start(out=wf, in_=src)
                    nc.vector.tensor_copy(out=we, in_=wf)
                tiles.append(we)
            return tiles

        def emit_m1(cg, we):
            """Interleaved natural mix for co-group cg -> stg[cg] (bf16)."""
            st = stgpool.tile([128, CI_T], BF16, tag="stg", name=f"stg{cg}")
            for off in range(0, CI_T, 512):
                w = min(512, CI_T - off)
                pm = ps.tile([128, 512], F32, tag="m1", bufs=M1_BUFS, name=f"pm1_{cg}_{off}")
                for g in range(2):
                    f32 = we[g].dtype == F32
                    nc.tensor.matmul(
                        pm[:, :w],
                        lhsT=Ls32[g] if f32 else Ls[g],
                        rhs=we[g][:, off : off + w],
                        start=(g == 0),
                        stop=(g == 1),
                    )
                nc.scalar.copy(st[:, off : off + w], pm[:, :w])
            stg[cg] = st

        def emit_m2(cg):
            """Transpose stg[cg] into per-sample conv lhsT layout."""
            b, cgr = cg // 4, cg % 4
            st_v = stg[cg].rearrange("p (ci t) -> p ci t", t=NTAP)
            for a in range(NCHUNK):
                for t0, tn in ((0, 4), (4, 4), (8, 1)):
                    pm = ps.tile([128, 512], F32, tag="m2", bufs=M2_BUFS, name=f"pm2_{cg}_{a}_{t0}")
                    for q in range(tn):
                        t = t0 + q
                        nc.tensor.matmul(
                            pm[:, q * 128 : (q + 1) * 128],
                            lhsT=st_v[:, a * 128 : (a + 1) * 128, t],
                            rhs=ident,
                            start=True,
                            stop=True,
                        )
                    nc.vector.tensor_copy(
                        out=wmT_v[b][
                            :,
                            a * NTAP + t0 : a * NTAP + t0 + tn,
                            :,
                            cgr * CGR : (cgr + 1) * CGR,
                        ],
                        in_=pm[:, : tn * 128].rearrange(
                            "p (q s j) -> p q s j", q=tn, s=B_LOC
                        ),
                    )

        def emit_xprep(s, row_chunks=1, via="pool"):
            """Zero-pad borders + load x[s] into tile interiors.

            via="pool": gpsimd f32->bf16 cast DMA straight into the interior
            (small-run penalty on the DMA pipe, but no engine work).
            via="poolstage"/"sync": f32 DMA to a staging tile on the pool /
            sync queue + deferred DVE cast copies into the interior (half the
            DMA-pipe cost; call the returned finish() later to place the DVE
            copies where they don't block other DVE work).
            row_chunks>1 splits the transfer/copies by row bands.
            """
            xps, xvs = [], []
            bounds = [H * i // row_chunks for i in range(row_chunks + 1)]
            for a in range(NCHUNK):
                xp = xppool.tile([128, PHW], BF16, tag="xpad", name=f"xp{s}_{a}")
                xv = xp.rearrange("c (r q) -> c r q", q=PW)
                nc.vector.memset(xv[:, 0:1, :], 0.0)
                nc.vector.memset(xv[:, PW - 1 : PW, :], 0.0)
                nc.vector.memset(xv[:, :, 0:1], 0.0)
                nc.vector.memset(xv[:, :, PW - 1 : PW], 0.0)
                xps.append(xp)
                xvs.append(xv)
            xpads[s] = xps
            if via in ("sync", "poolstage", "splitaq"):
                # "splitaq": ci-chunk a=0 stages on the sync ring, a=1 on
                # the qAct ring -- the two (slow, ~1/3-rate) HWDGE rings
                # pull in parallel.
                engs = {
                    "sync": (nc.sync, nc.sync),
                    "poolstage": (nc.gpsimd, nc.gpsimd),
                    "splitaq": (nc.sync, nc.scalar),
                }[via]
                stages = []
                for a in range(NCHUNK):
                    xs = wef32pool.tile(
                        [128, H * W], F32, tag="xs", bufs=2, name=f"xs{s}_{a}"
                    )
                    engs[a].dma_start(
                        out=xs, in_=x_d[s, a * 128 : (a + 1) * 128]
                    )
                    stages.append(xs.rearrange("c (h w) -> c h w", w=W))

                def finish():
                    for r0, r1 in zip(bounds, bounds[1:]):
                        for a in range(NCHUNK):
                            nc.vector.tensor_copy(
                                out=xvs[a][:, 1 + r0 : 1 + r1, 1 : W + 1],
                                in_=stages[a][:, r0:r1, :],
                            )

                return finish
            for r0, r1 in zip(bounds, bounds[1:]):
                for a in range(NCHUNK):
                    nc.gpsimd.dma_start(  # f32 -> bf16 cast DMA
                        out=xvs[a][:, 1 + r0 : 1 + r1, 1 : W + 1],
                        in_=x_d[s, a * 128 : (a + 1) * 128, r0:r1, :],
                    )
            return lambda: None

        def emit_conv(s, b, grps=None):
            # (tap, ci-chunk) outer over BAND_GRP concurrent psum bands:
            # consecutive matmuls share the stationary lhsT, so LDWEIGHTS
            # pressure drops ~BAND_GRP x.  grps optionally overrides the
            # band grouping (e.g. a 3-band first group so the first job can
            # start before the second half of x0 lands).
            # Output bands drain on the qAct (scalar) ring, keeping the
            # sync ring free for the x staging loads.
            xps = xpads[s]
            xvs = [xp.rearrange("c (r q) -> c r q", q=PW) for xp in xps]
            if grps is None:
                grps = [
                    BANDS[g0 : g0 + BAND_GRP]
                    for g0 in range(0, len(BANDS), BAND_GRP)
                ]
            for grp in grps:
                pcs = [
                    pcv.tile([128, BAND_MAX * W], F32, tag="cv",
                             name=f"pc{s}_{b}_{r0}")
                    for r0, _ in grp
                ]
                for i, (dy, dx, a) in enumerate(
                    (dy, dx, a)
                    for dy in range(KH)
                    for dx in range(KW)
                    for a in range(NCHUNK)
                ):
                    t = dy * KW + dx
                    off = s * CI_T + (a * NTAP + t) * 128
                    for (r0, nr), pc in zip(grp, pcs):
                        nc.tensor.matmul(
                            pc[:, : nr * W],
                            lhsT=wmT[b][:, off : off + 128],
                            rhs=xvs[a][:, r0 + dy : r0 + dy + nr, dx : dx + W],
                            start=(i == 0),
                            stop=(i == 2 * NTAP - 1),
                        )
                for (r0, nr), pc in zip(grp, pcs):
                    n = nr * W
                    ot = opool.tile([128, BAND_MAX * W], F32, tag="osb",
                                    name=f"ot{s}_{b}_{r0}")
                    nc.scalar.copy(ot[:, :n], pc[:, :n])
                    nc.scalar.dma_start(
                        out=o_d[s, b * 128 : (b + 1) * 128, r0 : r0 + nr, :],
                        in_=ot[:, :n],
                    )

        # ---- pipelined emission ----
        # A consumer of DMA X transitively waits for every DMA issued
        # earlier on X's queue, so the pool-queue order is the dependency
        # schedule: cg0..cg3 (everything conv(0,0) mixes from), then x0
        # staged f32 (half pipe cost), then the rest.  The deferred DVE pad
        # copies for x0 land after the M2 drains so they don't head-of-line
        # block the DVE queue.
        if PROBE == "conv":
            # timing probe: skip the whole mix; fill weights with memset
            for b in range(NCHUNK):
                nc.gpsimd.memset(wmT[b], 0.01)
            x0_fin = emit_xprep(0, row_chunks=2, via=X0_VIA)
            x0_fin()
            emit_conv(0, 0)
            emit_xprep(1)
            emit_conv(0, 1)
            emit_conv(1, 0)
            emit_xprep(2)
            emit_conv(1, 1)
            emit_conv(2, 0)
            emit_xprep(3)
            emit_conv(2, 1)
            emit_conv(3, 0)
            emit_conv(3, 1)
            return
        if PROBE == "mix":
            # timing probe: experts + M1 + M2 only
            wes = [None] * NCG
            for cg in range(NCG):
                wes[cg] = emit_expert_dma(cg)
            for cg in range(NCG):
                emit_m1(cg, wes[cg])
                emit_m2(cg)
            return
        if PROBE == "edma":
            # timing probe: expert cast-DMA stream alone (18.9MB f32 read)
            for cg in range(NCG):
                emit_expert_dma(cg)
            return
        if PROBE == "edmasync":
            # timing probe: expert f32 loads on the HWDGE (sync) queue only
            for cg in range(NCG):
                for g in range(2):
                    wf = wef32pool.tile(
                        [128, CI_T], F32, tag="wef32", bufs=2, name=f"ws{cg}_{g}"
                    )
                    nc.sync.dma_start(
                        out=wf,
                        in_=w_v[g * 4 : (g + 1) * 4,
                                cg * CGR : (cg + 1) * CGR, :],
                    )
            return
        if PROBE == "edmasplit":
            # timing probe: expert loads split across pool(cast)/sync(f32)
            for cg in range(NCG):
                if cg % 2 == 0:
                    emit_expert_dma(cg)
                else:
                    for g in range(2):
                        wf = wef32pool.tile(
                            [128, CI_T], F32, tag="wef32", name=f"wf{cg}_{g}"
                        )
                        nc.sync.dma_start(
                            out=wf,
                            in_=w_v[g * 4 : (g + 1) * 4,
                                    cg * CGR : (cg + 1) * CGR, :],
                        )
            return
        # ---- pipelined emission ----
        # Measured ring rates: gpsimd/SWDGE cast-DMA ~303 GB/s; each HWDGE
        # ring (sync, qAct) only ~101 GB/s.  So the expert bank (18.9MB)
        # streams exclusively on the fast gpsimd ring (f32->bf16 cast for
        # free), x0..x2 stage f32 on sync (a=0) + qAct (a=1) concurrently,
        # x3 rides the gpsimd ring's slack after the expert bank, and
        # outputs drain on qAct behind the x pulls.
        x0_fin = emit_xprep(0, via="splitaq", row_chunks=2)
        wes = [None] * NCG
        wes[0] = emit_expert_dma(0)
        wes[1] = emit_expert_dma(1)
        x1_fin = emit_xprep(1, via="splitaq")
        wes[2] = emit_expert_dma(2)
        emit_m1(0, wes[0])
        wes[3] = emit_expert_dma(3)
        emit_m1(1, wes[1])
        emit_m2(0)
        x0_fin()
        x2_fin = emit_xprep(2, via="splitaq")
        emit_m1(2, wes[2])
        emit_m2(1)
        wes[4] = emit_expert_dma(4)
        emit_m1(3, wes[3])
        emit_m2(2)
        wes[5] = emit_expert_dma(5)
        emit_m2(3)

        wes[6] = emit_expert_dma(6)
        wes[7] = emit_expert_dma(7)

        emit_conv(0, 0)
        x1_fin()
        emit_conv(1, 0)
        x2_fin()
        x3_fin = emit_xprep(3, via="poolstage")
        emit_conv(2, 0)
        x3_fin()

        emit_m1(4, wes[4])
        emit_m2(4)
        emit_m1(5, wes[5])
        emit_m2(5)
        emit_m1(6, wes[6])
        emit_m2(6)
        emit_m1(7, wes[7])
        emit_m2(7)

        emit_conv(3, 0)
        emit_conv(0, 1)
        emit_conv(1, 1)
        emit_conv(2, 1)
        emit_conv(3, 1)


_CACHED_NC = None


def kernel(x, routing_weights, weight_experts, *, trace=False):
    global _CACHED_NC
    x = np.ascontiguousarray(np.asarray(x, dtype=np.float32))
    routing_weights = np.ascontiguousarray(
        np.asarray(routing_weights, dtype=np.float32)
    )
    weight_experts = np.ascontiguousarray(np.asarray(weight_experts, dtype=np.float32))

    if _CACHED_NC is None:
        _CACHED_NC = build_program()
    nc = _CACHED_NC

    in_maps = []
    for c in range(N_CORES):
        lo, hi = c * B_LOC, (c + 1) * B_LOC
        in_maps.append(
            {
                "x_loc": x[lo:hi],
                "r_loc": routing_weights[lo:hi],
                "experts": weight_experts,
            }
        )

    res = bass_utils.run_bass_kernel_spmd(
        nc, in_maps, core_ids=list(range(N_CORES)), trace=trace
    )

    out = np.empty((B, C, H, W), dtype=np.float32)
    for c in range(N_CORES):
        out[c * B_LOC : (c + 1) * B_LOC] = res.results[c]["out_loc"]
    if trace:
        return out, res
    return out



# revision 17
# speedup vs baseline: 1.1250x; 1.1250x over previous
# CondConv2d Trainium2 kernel (Bass/Tile), data-parallel over batch on 8 cores.
#
# Problem (hardcoded):
#   x:               [32, 256, 56, 56] f32
#   routing_weights: [32, 8] f32
#   weight_experts:  [8, 589824] f32      (589824 = 256*256*3*3, co-major)
#   out:             [32, 256, 56, 56] f32
#   out[b] = conv2d(x[b], (routing[b] @ experts).reshape(256,256,3,3), pad=1)
#
# Sharding: 4 samples per core; expert bank replicated.
#
# Per-core kernel (pipelined):
#   M1 (interleaved mix): natural-layout mixed weights for all 4 samples via
#     matmuls with a block-diagonal routing lhsT  L[(e,j'),(s,j)] = r[s,e]*I32,
#     contraction over (4 experts x 32 co-rows); two accumulating matmuls
#     (expert groups 0-3 / 4-7) per psum tile.  8 co-groups (cg) of 32 rows.
#   M2 (transpose): per (cg, ci-chunk, tap): one identity-rhs matmul
#     transposes the (s,j)-packed natural rows into [ci, (s,j)] columns;
#     strided DVE copies scatter them into per-sample conv lhsT layout.
#   Conv: 9 taps x 2 ci-chunks bf16 matmuls (N=448, 8-row bands) into PSUM
#     over zero-padded [128, 58*58] bf16 input tiles; (tap, ci-chunk) is the
#     outer loop over 4 concurrent band psums so consecutive matmuls share
#     the stationary operand (fewer LDWEIGHTS reloads).
#   Data movement (measured per-queue rates differ ~2x): the expert bank
#     streams as f32->bf16 cast DMAs on the gpsimd/SWDGE queue ordered
#     cg0..3 (conv(0,0)'s weights) then x0 then cg4..7; x rides f32 staging
#     tiles + deferred DVE cast/pad copies (x1..x3 on the HWDGE queue);
#     conv jobs run b-major (all samples' co-half 0 first) so the second
#     half of the expert bank has maximal slack; outputs drain on HWDGE.
#   A consumer of DMA X transitively waits for all earlier DMAs on X's
#   queue, so per-queue emission order is need-order.

import numpy as np

import concourse.bass as bass
import concourse.mybir as mybir
import concourse.tile as tile
from concourse import bacc, bass_utils

# ---- problem constants ----
B = 32
E = 8
C = 256  # CIN = COUT
H = W = 56
KH = KW = 3
NPARAM = C * C * KH * KW  # 589824
N_CORES = 8
B_LOC = B // N_CORES  # 4 samples per core

PW = W + 2  # 58
PHW = PW * PW  # 3364
CI_T = C * KH * KW  # 2304  (ci*9 + t) extent per co row
NCHUNK = C // 128  # 2 chunks of 128 for both ci and co
ROWS = 8  # output rows per conv matmul tile
NSP = H // ROWS  # 7 spatial tiles
NTAP = KH * KW  # 9
NCG = 8  # co-groups of 32 rows
CGR = 32  # rows per co-group

F32 = mybir.dt.float32
BF16 = mybir.dt.bfloat16

X0_VIA = "poolstage"  # how x[0] is loaded: pool | sync | poolstage
PROBE = None  # None | "conv" | "mix"
M1_BUFS = 2
M2_BUFS = 2
BAND_MAX = 8
BANDS = [(i * 8, 8) for i in range(7)]
BAND_GRP = 4  # bands accumulated concurrently (shared stationary weights)


def build_program(reps=1, hw_loop=False):
    nc = bacc.Bacc(
        "TRN2",
        target_bir_lowering=False,
        debug=False,
        enable_asserts=False,
    )

    x_d = nc.dram_tensor("x_loc", [B_LOC, C, H, W], F32, kind="ExternalInput").ap()
    r_d = nc.dram_tensor("r_loc", [B_LOC, E], F32, kind="ExternalInput").ap()
    w_d = nc.dram_tensor("experts", [E, NPARAM], F32, kind="ExternalInput").ap()
    o_d = nc.dram_tensor("out_loc", [B_LOC, C, H, W], F32, kind="ExternalOutput").ap()

    with tile.TileContext(nc) as tc:
        if hw_loop and reps > 1:
            with tc.For_i(0, reps):
                _emit_body(nc, tc, x_d, r_d, w_d, o_d)
        else:
            for _ in range(reps):
                _emit_body(nc, tc, x_d, r_d, w_d, o_d)

    nc.compile()
    return nc


def _emit_body(nc, tc, x_d, r_d, w_d, o_d):
    w_v = w_d.rearrange("e (co r) -> e co r", r=CI_T)  # [8, 256, 2304]

    with (
        tc.tile_pool(name="const", bufs=1) as cpool,
        tc.tile_pool(name="wexp", bufs=8) as wepool,
        tc.tile_pool(name="wef32", bufs=4) as wef32pool,
        tc.tile_pool(name="stg", bufs=2) as stgpool,
        tc.tile_pool(name="xpad", bufs=8) as xppool,
        tc.tile_pool(name="osb", bufs=7) as opool,
        tc.tile_pool(name="ps", bufs=3, space="PSUM") as ps,
        tc.tile_pool(name="cv", bufs=BAND_GRP, space="PSUM") as pcv,
    ):
        # ---- prep: broadcast routing weights to all partitions ----
        r_sb = cpool.tile([1, B_LOC * E], F32, tag="r_sb")
        nc.sync.dma_start(out=r_sb, in_=r_d.rearrange("s e -> (s e)")[None, :])
        ones = cpool.tile([1, 128], F32, tag="ones")
        nc.vector.memset(ones, 1.0)
        r_ps = ps.tile([128, B_LOC * E], F32, tag="m2", bufs=M2_BUFS)
        nc.tensor.matmul(r_ps, lhsT=ones, rhs=r_sb, start=True, stop=True)
        r_bc = cpool.tile([128, B_LOC * E], F32, tag="r_bc")
        nc.scalar.copy(r_bc, r_ps)

        # ---- identity (bf16) ----
        ident = cpool.tile([128, 128], BF16, tag="ident")
        nc.gpsimd.memset(ident, 0.0)
        nc.gpsimd.affine_select(
            out=ident,
            in_=ident,
            compare_op=mybir.AluOpType.not_equal,
            fill=1.0,
            base=0,
            pattern=[[-1, 128]],
            channel_multiplier=1,
        )

        # ---- block-diagonal routing lhsT for the interleaved mix ----
        # L[g][e_rel*32+j', s*32+j] = r[s, g*4+e_rel] * (j==j')
        Ls, Ls32 = [], []
        for g in range(2):
            for dt, lst in ((BF16, Ls), (F32, Ls32)):
                sfx = "" if dt is BF16 else "f"
                L_g = cpool.tile([128, 128], dt, tag=f"L{g}{sfx}", name=f"L_g{g}{sfx}")
                nc.vector.memset(L_g, 0.0)
                for er in range(4):
                    pr = slice(er * CGR, (er + 1) * CGR)
                    for s in range(B_LOC):
                        c = s * E + g * 4 + er
                        nc.vector.tensor_scalar_mul(
                            L_g[pr, s * CGR : (s + 1) * CGR],
                            ident[pr, pr.start : pr.stop],
                            r_bc[pr, c : c + 1],
                        )
                lst.append(L_g)

        # per-sample transposed mixed weights, conv lhsT layout:
        # wmT[b][:, s*2304 + (a*9+t)*128 + co'] = W_s[b*128+co', a*128+ci, t]
        wmT = []
        for b in range(NCHUNK):
            wt = cpool.tile([128, B_LOC * CI_T], BF16, tag=f"wmT{b}")
            wmT.append(wt)
        # [128, 18, 4, 128] views: (a*9+t, s, co')
        wmT_v = [
            wt.rearrange("p (s at c) -> p at s c", s=B_LOC, at=NCHUNK * NTAP, c=128)
            for wt in wmT
        ]

        stg = [None] * NCG
        xpads = [None] * B_LOC

        def emit_expert_dma(cg, via="pool"):
            """DMA expert rows cg*32..cg*32+32 for both expert groups, bf16.

            via="pool": gpsimd cast DMA (SWDGE queue).
            via="sync": f32 DMA on the HWDGE queue + DVE cast copy, so the
            expert stream is split across two DMA queues.
            """
            tiles = []
            for g in range(2):
                we = wepool.tile([128, CI_T], BF16, tag="wexp", name=f"we{cg}_{g}")
                src = w_v[g * 4 : (g + 1) * 4, cg * CGR : (cg + 1) * CGR, :]
                if via == "pool":
                    nc.gpsimd.dma_start(out=we, in_=src)  # f32 -> bf16 cast DMA
                elif via == "poolf32":
                    # plain f32 load (full DMA rate); M1 consumes f32 directly
                    wf = wef32pool.tile(
                        [128, CI_T], F32, tag="wef32", name=f"wef{cg}_{g}"
                    )
                    nc.gpsimd.dma_start(out=wf, in_=src)
                    tiles.append(wf)
                    continue
                else:
                    wf = wef32pool.tile(
                        [128, CI_T], F32, tag="wef32", name=f"wef{cg}_{g}"
                    )
                    nc.sync.dma_start(out=wf, in_=src)
                    nc.vector.tensor_copy(out=we, in_=wf)
                tiles.append(we)
            return tiles

        def emit_m1(cg, we):
            """Interleaved natural mix for co-group cg -> stg[cg] (bf16)."""
            st = stgpool.tile([128, CI_T], BF16, tag="stg", name=f"stg{cg}")
            for off in range(0, CI_T, 512):
                w = min(512, CI_T - off)
                pm = ps.tile([128, 512], F32, tag="m1", bufs=M1_BUFS, name=f"pm1_{cg}_{off}")
                for g in range(2):
                    f32 = we[g].dtype == F32
                    nc.tensor.matmul(
                        pm[:, :w],
                        lhsT=Ls32[g] if f32 else Ls[g],
                        rhs=we[g][:, off : off + w],
                        start=(g == 0),
                        stop=(g == 1),
                    )
                nc.scalar.copy(st[:, off : off + w], pm[:, :w])
            stg[cg] = st

        def emit_m2(cg):
            """Transpose stg[cg] into per-sample conv lhsT layout."""
            b, cgr = cg // 4, cg % 4
            st_v = stg[cg].rearrange("p (ci t) -> p ci t", t=NTAP)
            for a in range(NCHUNK):
                for t0, tn in ((0, 4), (4, 4), (8, 1)):
                    pm = ps.tile([128, 512], F32, tag="m2", bufs=M2_BUFS, name=f"pm2_{cg}_{a}_{t0}")
                    for q in range(tn):
                        t = t0 + q
                        nc.tensor.matmul(
                            pm[:, q * 128 : (q + 1) * 128],
                            lhsT=st_v[:, a * 128 : (a + 1) * 128, t],
                            rhs=ident,
                            start=True,
                            stop=True,
                        )
                    nc.vector.tensor_copy(
                        out=wmT_v[b][
                            :,
                            a * NTAP + t0 : a * NTAP + t0 + tn,
                            :,
                            cgr * CGR : (cgr + 1) * CGR,
                        ],
                        in_=pm[:, : tn * 128].rearrange(
                            "p (q s j) -> p q s j", q=tn, s=B_LOC
                        ),
                    )

        def emit_xprep(s, row_chunks=1, via="pool"):
            """Zero-pad borders + load x[s] into tile interiors.

            via="pool": gpsimd f32->bf16 cast DMA straight into the interior
            (small-run penalty on the DMA pipe, but no engine work).
            via="poolstage"/"sync": f32 DMA to a staging tile on the pool /
            sync queue + deferred DVE cast copies into the interior (half the
            DMA-pipe cost; call the returned finish() later to place the DVE
            copies where they don't block other DVE work).
            row_chunks>1 splits the transfer/copies by row bands.
            """
            xps, xvs = [], []
            bounds = [H * i // row_chunks for i in range(row_chunks + 1)]
            for a in range(NCHUNK):
                xp = xppool.tile([128, PHW], BF16, tag="xpad", name=f"xp{s}_{a}")
                xv = xp.rearrange("c (r q) -> c r q", q=PW)
                nc.vector.memset(xv[:, 0:1, :], 0.0)
                nc.vector.memset(xv[:, PW - 1 : PW, :], 0.0)
                nc.vector.memset(xv[:, :, 0:1], 0.0)
                nc.vector.memset(xv[:, :, PW - 1 : PW], 0.0)
                xps.append(xp)
                xvs.append(xv)
            xpads[s] = xps
            if via in ("sync", "poolstage"):
                eng = nc.sync if via == "sync" else nc.gpsimd
                stages = []
                for a in range(NCHUNK):
                    xs = wef32pool.tile(
                        [128, H * W], F32, tag="xs", bufs=2, name=f"xs{s}_{a}"
                    )
                    eng.dma_start(
                        out=xs, in_=x_d[s, a * 128 : (a + 1) * 128]
                    )
                    stages.append(xs.rearrange("c (h w) -> c h w", w=W))

                def finish():
                    for r0, r1 in zip(bounds, bounds[1:]):
                        for a in range(NCHUNK):
                            nc.vector.tensor_copy(
                                out=xvs[a][:, 1 + r0 : 1 + r1, 1 : W + 1],
                                in_=stages[a][:, r0:r1, :],
                            )

                return finish
            for r0, r1 in zip(bounds, bounds[1:]):
                for a in range(NCHUNK):
                    nc.gpsimd.dma_start(  # f32 -> bf16 cast DMA
                        out=xvs[a][:, 1 + r0 : 1 + r1, 1 : W + 1],
                        in_=x_d[s, a * 128 : (a + 1) * 128, r0:r1, :],
                    )
            return lambda: None

        def emit_conv(s, b, grps=None):
            # (tap, ci-chunk) outer over BAND_GRP concurrent psum bands:
            # consecutive matmuls share the stationary lhsT, so LDWEIGHTS
            # pressure drops ~BAND_GRP x.  grps optionally overrides the
            # band grouping (e.g. a 3-band first group so the first job can
            # start before the second half of x0 lands).
            xps = xpads[s]
            xvs = [xp.rearrange("c (r q) -> c r q", q=PW) for xp in xps]
            if grps is None:
                grps = [
                    BANDS[g0 : g0 + BAND_GRP]
                    for g0 in range(0, len(BANDS), BAND_GRP)
                ]
            for grp in grps:
                pcs = [
                    pcv.tile([128, BAND_MAX * W], F32, tag="cv",
                             name=f"pc{s}_{b}_{r0}")
                    for r0, _ in grp
                ]
                for i, (dy, dx, a) in enumerate(
                    (dy, dx, a)
                    for dy in range(KH)
                    for dx in range(KW)
                    for a in range(NCHUNK)
                ):
                    t = dy * KW + dx
                    off = s * CI_T + (a * NTAP + t) * 128
                    for (r0, nr), pc in zip(grp, pcs):
                        nc.tensor.matmul(
                            pc[:, : nr * W],
                            lhsT=wmT[b][:, off : off + 128],
                            rhs=xvs[a][:, r0 + dy : r0 + dy + nr, dx : dx + W],
                            start=(i == 0),
                            stop=(i == 2 * NTAP - 1),
                        )
                for (r0, nr), pc in zip(grp, pcs):
                    n = nr * W
                    ot = opool.tile([128, BAND_MAX * W], F32, tag="osb",
                                    name=f"ot{s}_{b}_{r0}")
                    nc.scalar.copy(ot[:, :n], pc[:, :n])
                    # co-chunk 0 drains on sync behind the x pulls; chunk 1
                    # on the otherwise-idle qAct ring
                    (nc.sync if b == 0 else nc.scalar).dma_start(
                        out=o_d[s, b * 128 : (b + 1) * 128, r0 : r0 + nr, :],
                        in_=ot[:, :n],
                    )

        # ---- pipelined emission ----
        # A consumer of DMA X transitively waits for every DMA issued
        # earlier on X's queue, so the pool-queue order is the dependency
        # schedule: cg0..cg3 (everything conv(0,0) mixes from), then x0
        # staged f32 (half pipe cost), then the rest.  The deferred DVE pad
        # copies for x0 land after the M2 drains so they don't head-of-line
        # block the DVE queue.
        if PROBE == "conv":
            # timing probe: skip the whole mix; fill weights with memset
            for b in range(NCHUNK):
                nc.gpsimd.memset(wmT[b], 0.01)
            x0_fin = emit_xprep(0, row_chunks=2, via=X0_VIA)
            x0_fin()
            emit_conv(0, 0)
            emit_xprep(1)
            emit_conv(0, 1)
            emit_conv(1, 0)
            emit_xprep(2)
            emit_conv(1, 1)
            emit_conv(2, 0)
            emit_xprep(3)
            emit_conv(2, 1)
            emit_conv(3, 0)
            emit_conv(3, 1)
            return
        if PROBE == "mix":
            # timing probe: experts + M1 + M2 only
            wes = [None] * NCG
            for cg in range(NCG):
                wes[cg] = emit_expert_dma(cg)
            for cg in range(NCG):
                emit_m1(cg, wes[cg])
                emit_m2(cg)
            return
        if PROBE == "edma":
            # timing probe: expert cast-DMA stream alone (18.9MB f32 read)
            for cg in range(NCG):
                emit_expert_dma(cg)
            return
        if PROBE == "edmasync":
            # timing probe: expert f32 loads on the HWDGE (sync) queue only
            for cg in range(NCG):
                for g in range(2):
                    wf = wef32pool.tile(
                        [128, CI_T], F32, tag="wef32", bufs=2, name=f"ws{cg}_{g}"
                    )
                    nc.sync.dma_start(
                        out=wf,
                        in_=w_v[g * 4 : (g + 1) * 4,
                                cg * CGR : (cg + 1) * CGR, :],
                    )
            return
        if PROBE == "edmasplit":
            # timing probe: expert loads split across pool(cast)/sync(f32)
            for cg in range(NCG):
                if cg % 2 == 0:
                    emit_expert_dma(cg)
                else:
                    for g in range(2):
                        wf = wef32pool.tile(
                            [128, CI_T], F32, tag="wef32", name=f"wf{cg}_{g}"
                        )
                        nc.sync.dma_start(
                            out=wf,
                            in_=w_v[g * 4 : (g + 1) * 4,
                                    cg * CGR : (cg + 1) * CGR, :],
                        )
            return
        # x0: pad tiles + borders up front; f32 staging DMAs row-split
        # around cg2/cg3 on the pool queue so the first half of x0 lands
        # early; DVE cast/pad copies deferred behind the M2 drains.
        R0 = 28
        x0ps, x0vs, x0ss = [], [], []
        for a in range(NCHUNK):
            xp = xppool.tile([128, PHW], BF16, tag="xpad", name=f"xp0_{a}")
            xv = xp.rearrange("c (r q) -> c r q", q=PW)
            nc.vector.memset(xv[:, 0:1, :], 0.0)
            nc.vector.memset(xv[:, PW - 1 : PW, :], 0.0)
            nc.vector.memset(xv[:, :, 0:1], 0.0)
            nc.vector.memset(xv[:, :, PW - 1 : PW], 0.0)
            xs = wef32pool.tile(
                [128, H * W], F32, tag="xs", bufs=2, name=f"xs0_{a}"
            )
            x0ps.append(xp)
            x0vs.append(xv)
            x0ss.append(xs)
        xpads[0] = x0ps

        def x0_dma(r0, r1):
            for a in range(NCHUNK):
                nc.gpsimd.dma_start(
                    out=x0ss[a][:, r0 * W : r1 * W],
                    in_=x_d[0, a * 128 : (a + 1) * 128, r0:r1, :],
                )

        def x0_copy(r0, r1):
            for a in range(NCHUNK):
                nc.vector.tensor_copy(
                    out=x0vs[a][:, 1 + r0 : 1 + r1, 1 : W + 1],
                    in_=x0ss[a].rearrange("c (h w) -> c h w", w=W)[:, r0:r1, :],
                )

        wes = [None] * NCG
        wes[0] = emit_expert_dma(0)
        wes[1] = emit_expert_dma(1)
        x0_dma(0, R0)
        wes[2] = emit_expert_dma(2)
        emit_m1(0, wes[0])
        wes[3] = emit_expert_dma(3)
        emit_m1(1, wes[1])
        emit_m2(0)
        x0_dma(R0, H)
        emit_m1(2, wes[2])
        emit_m2(1)
        wes[4] = emit_expert_dma(4)
        emit_m1(3, wes[3])
        emit_m2(2)
        x0_copy(0, R0)
        wes[5] = emit_expert_dma(5)
        emit_m2(3)
        x0_copy(R0, H)

        wes[6] = emit_expert_dma(6)
        wes[7] = emit_expert_dma(7)
        x1_fin = emit_xprep(1, via="sync")

        emit_conv(0, 0, grps=[BANDS[0:3], BANDS[3:7]])
        x1_fin()
        x2_fin = emit_xprep(2, via="sync")
        emit_conv(1, 0)
        x2_fin()
        x3_fin = emit_xprep(3, via="sync")
        emit_conv(2, 0)
        x3_fin()

        emit_m1(4, wes[4])
        emit_m2(4)
        emit_m1(5, wes[5])
        emit_m2(5)
        emit_m1(6, wes[6])
        emit_m2(6)
        emit_m1(7, wes[7])
        emit_m2(7)

        emit_conv(3, 0)
        emit_conv(0, 1)
        emit_conv(1, 1)
        emit_conv(2, 1)
        emit_conv(3, 1)


_CACHED_NC = None


def kernel(x, routing_weights, weight_experts, *, trace=False):
    global _CACHED_NC
    x = np.ascontiguousarray(np.asarray(x, dtype=np.float32))
    routing_weights = np.ascontiguousarray(
        np.asarray(routing_weights, dtype=np.float32)
    )
    weight_experts = np.ascontiguousarray(np.asarray(weight_experts, dtype=np.float32))

    if _CACHED_NC is None:
        _CACHED_NC = build_program()
    nc = _CACHED_NC

    in_maps = []
    for c in range(N_CORES):
        lo, hi = c * B_LOC, (c + 1) * B_LOC
        in_maps.append(
            {
                "x_loc": x[lo:hi],
                "r_loc": routing_weights[lo:hi],
                "experts": weight_experts,
            }
        )

    res = bass_utils.run_bass_kernel_spmd(
        nc, in_maps, core_ids=list(range(N_CORES)), trace=trace
    )

    out = np.empty((B, C, H, W), dtype=np.float32)
    for c in range(N_CORES):
        out[c * B_LOC : (c + 1) * B_LOC] = res.results[c]["out_loc"]
    if trace:
        return out, res
    return out

